# revision 35
# baseline (speedup 1.0000x reference)
"""Fused CE + supervised-contrastive loss on 8 Trainium2 NeuronCores.

Math (reference semantics):
  ce   = -mean_i log_softmax(input)[i, y_i]
  sim  = (X @ X.T) / tau, diag excluded
  lse_i = logsumexp_{k!=i} sim[i,k]
  possum_i = (x_i . S_{y_i} - ||x_i||^2)/tau,  S_c = sum_{k: y_k=c} x_k
  per_i = lse_i - possum_i/n_pos_i  (0 if n_pos_i == 0)
  loss = (1-lmbd)*ce + lmbd * sum_i per_i

Only the O(N^2) term (the per-row exp-sums se_i = sum_k e^{sim-40})
runs on device; everything O(N*C) -- the CE term, class sums /
positive-pair dot products, and the final ln(se)/per_i combine -- is
cheap host numpy.  Each core owns 1024 rows (batch shard) and holds the
full X^T (replicated, bf16) so the sim block needs no communication
(the ncfw AllReduce measures 44us trigger-to-start latency on this
platform -- longer than the kernel -- so no collective is used).

The N^2 elements must leave PSUM through the only two engines with PSUM
read ports (ACT + DVE), so the kernel is drain-bound:
  - PE: sim matmuls emitted as 64x128 row-tile PAIRS, tile_position
    (0,0)/(64,0) alternating; disjoint row-groups execute concurrently.
  - ACT drains 3x[128,1536] chunks per block: exp(s-40) with fused
    accum_out row sums.
  - DVE drains 7x[128,512] chunks per block with a bit-hack exp:
    u16 = max(A*s, 0) gives the bf16 bits of ~e^(s - 127*ln2); the
    max-with-0 zeroes the -1e4 diag spikes and the s<0 tail (true
    contributions < e^-88: exactly the bf16 underflow).
  - The bits second pass: DVE pairwise bf16 folds (2x mode) down to
    448 cols, then one small Copy+accum on ACT.
Input DMAs: a single "head bundle" (xbtd | diag operands | xt2d cols
0:1536) lands first so matmuls start as early as possible.

Self-exclusion: X^T is rotated by -1024*core so row p of block b has
its self-column at local column b*128+p; two 64x128-tiled
accumulate-matmuls add -1e4 there before the drain (pure SPMD).

Outputs per core: [128, 40] f32 = per-(row-block, chunk) partial exp
sums (3 ACT slots + 2 bits slots per block).  Host combines in f64.
"""

import numpy as np

N, C = 8192, 64
NCORES = 8
RPC = N // NCORES          # rows per core (1024)
P = 128                    # partitions per row-block
NBLK = RPC // P            # 8 row blocks per core
TAU = 0.5
LMBD = 0.5

SH = 40.0                          # ACT-chunk logsumexp shift
A_EXP = 128.0 / float(np.log(2.0))         # 184.6627...
# DVE chunks compute max(A*s, 0) -> bf16 bits of e^(s - 127*ln2), i.e. an
# implied shift of 88.03; SCALE48 rescales their row sums to the ACT shift
# (the /1.042 centers the piecewise-linear 2^x hack's +0..8.6% bias).
SH_DVE = 127.0 * float(np.log(2.0))
SCALE48 = float(np.exp(SH_DVE - SH) / 1.042)

# per-block drain plan: (kind, start_col, width); A->ACT exp+accum,
# D->DVE bit-hack exp.  A: 3x1536, D: 7x512 -> 8192 total.
CHUNK_PLAN = [
    ("A", 0, 1536), ("D", 4608, 512), ("D", 5120, 512),
    ("A", 1536, 1536), ("D", 5632, 512), ("D", 6144, 512),
    ("A", 3072, 1536), ("D", 6656, 512), ("D", 7168, 512),
    ("D", 7680, 512),
]
# last block: A chunks first so ACT's final drain overlaps the D tail;
# bits folded in two pieces so the serial tail is small.
CHUNK_PLAN_LAST = [
    ("A", 0, 1536), ("A", 1536, 1536), ("A", 3072, 1536),
    ("D", 4608, 512), ("D", 5120, 512), ("D", 5632, 512),
    ("D", 6144, 512), ("D", 6656, 512), ("D", 7168, 512),
    ("D", 7680, 512),
]
N_ACT = sum(1 for k, _, _ in CHUNK_PLAN if k == "A")   # 3
N_DVE = sum(1 for k, _, _ in CHUNK_PLAN if k == "D")   # 7
NPIECE = 2
O_ED = NBLK * N_ACT        # esumD slot base inside the out tile (24)
OUT_W = O_ED + NPIECE * NBLK   # 40

_CACHE = {}


def _build():
    from contextlib import ExitStack

    import concourse.bass as bass
    import concourse.tile as tile
    from concourse import bacc, mybir

    f32 = mybir.dt.float32
    u16 = mybir.dt.uint16
    bf16 = mybir.dt.bfloat16
    AF = mybir.ActivationFunctionType
    ALU = mybir.AluOpType

    nc = bacc.Bacc(
        "TRN2",
        target_bir_lowering=False,
        debug=False,
        num_devices=NCORES,
    )

    # head bundle: xbtd | eyeneg+idn64 (padded to 128 rows) | xt2d[:, 0:1536]
    # -- everything block 0's first chunk needs, in ONE dma so the first
    # matmul isn't gated on multiple serialized transfers.
    HB_OSE = RPC                       # 1024
    HB_XT = RPC + 2 * P + C            # 1344
    HB_W = HB_XT + 1536                # 2880
    hd_d = nc.dram_tensor("headb", [P, HB_W], bf16, kind="ExternalInput")
    # xt2d columns 1536:8192
    xt2d_d = nc.dram_tensor("xt2d", [P, N - 1536], bf16, kind="ExternalInput")
    out_d = nc.dram_tensor("out", [P, OUT_W], f32, kind="ExternalOutput")

    def emit(tc, ctx):
        const = ctx.enter_context(tc.tile_pool(name="const", bufs=1))
        psA = ctx.enter_context(tc.tile_pool(name="psA", bufs=2, space="PSUM"))
        psD = ctx.enter_context(tc.tile_pool(name="psD", bufs=2, space="PSUM"))
        bitsp = ctx.enter_context(tc.tile_pool(name="bits", bufs=2))
        scr = ctx.enter_context(tc.tile_pool(name="scr", bufs=2))
        stats = ctx.enter_context(tc.tile_pool(name="stats", bufs=1))

        # ---- input DMAs: head bundle first, the rest behind it ----
        hd_sb = const.tile([P, HB_W], bf16)
        nc.sync.dma_start(hd_sb[:], hd_d.ap())
        O_EYE, O_IDN = HB_OSE, HB_OSE + 2 * P
        xt2d_sb = const.tile([P, N - 1536], bf16)
        nc.sync.dma_start(xt2d_sb[:, 0:3072], xt2d_d.ap()[:, 0:3072])
        nc.sync.dma_start(xt2d_sb[:, 3072:6656], xt2d_d.ap()[:, 3072:6656])

        def rhs_ap(h, c0, w):
            """xt2d columns c0:c0+w live in the head bundle (<1536) or the
            xt2d tile (>=1536); chunk pieces never straddle the boundary."""
            if c0 < 1536:
                assert c0 + w <= 1536
                return hd_sb[h : h + 64, HB_XT + c0 : HB_XT + c0 + w]
            return xt2d_sb[h : h + 64, c0 - 1536 : c0 - 1536 + w]

        # ---- persistent stat tiles ----
        nshift = stats.tile([P, 1], f32)
        nc.vector.memset(nshift[:], -SH)
        res = stats.tile([P, OUT_W], f32)
        nc.vector.memset(res[:, O_ED:OUT_W], 0.0)

        # ---- the O(N^2) drain ----
        toggle = [0]

        def sim_mms(ps, b, c0, width):
            """row-tiled sim matmuls: 512-col pieces, alternating PE halves."""
            lo = b * P
            for j in range(width // 512):
                h = 64 * toggle[0]
                toggle[0] ^= 1
                nc.tensor.matmul(
                    ps[:, j * 512 : (j + 1) * 512],
                    lhsT=hd_sb[h : h + 64, lo : lo + P],
                    rhs=rhs_ap(h, c0 + j * 512, 512),
                    start=True,
                    stop=True,
                )

        def diag_mms(ps, b):
            """kill self-similarity: -1e4 onto local cols b*128..+127.
            Both pieces on tile (0,0) so they serialize (concurrent row-tiles
            must not write the same PSUM bank)."""
            d0 = b * P
            nc.tensor.matmul(
                ps[:, d0 : d0 + 64],
                lhsT=hd_sb[0:C, O_EYE : O_EYE + P],
                rhs=hd_sb[0:C, O_IDN : O_IDN + C],
                start=False, stop=True, skip_group_check=True,
            )
            nc.tensor.matmul(
                ps[:, d0 + 64 : d0 + P],
                lhsT=hd_sb[0:C, O_EYE + P : O_EYE + 2 * P],
                rhs=hd_sb[0:C, O_IDN : O_IDN + C],
                start=False, stop=True, skip_group_check=True,
            )

        def bits_sum(bits, lo, hi, slot, b):
            """fold bits[lo:hi] pairwise (bf16 TT, 2x mode) down to <=512,
            then Copy+accum the remnant on ACT into the esumD slot."""
            w = hi - lo
            cur, c0 = bits, lo
            while w > 512:
                nxt = scr.tile([P, w // 2], bf16, tag="fold")
                nc.vector.tensor_add(
                    nxt[:], cur[:, c0 : c0 + w // 2], cur[:, c0 + w // 2 : c0 + w]
                )
                cur, c0, w = nxt, 0, w // 2
            t = scr.tile([P, w], bf16, tag="crdump")
            col = O_ED + slot * NBLK + b
            nc.scalar.activation(
                t[:], cur[:, c0 : c0 + w], AF.Copy,
                accum_out=res[:, col : col + 1],
            )

        for b in range(NBLK):
            last = b == NBLK - 1
            di = 0
            bits = bitsp.tile([P, N_DVE * 512], bf16, tag="bits")
            plan = CHUNK_PLAN_LAST if last else CHUNK_PLAN
            for kind, c0, width in plan:
                if kind == "A":
                    ps = psA.tile([P, 1536], f32, tag="ps")
                    sim_mms(ps, b, c0, width)
                    if c0 == 0:
                        diag_mms(ps, b)
                    aidx = c0 // 1536
                    nc.scalar.activation(
                        ps[:], ps[:], AF.Exp, bias=nshift[:],
                        accum_out=res[:, b * N_ACT + aidx : b * N_ACT + aidx + 1],
                    )
                else:
                    ps = psD.tile([P, 512], f32, tag="ps")
                    sim_mms(ps, b, c0, 512)
                    nc.vector.tensor_scalar(
                        out=bits[:, di * 512 : (di + 1) * 512].bitcast(u16),
                        in0=ps[:],
                        scalar1=A_EXP, scalar2=0.0,
                        op0=ALU.mult, op1=ALU.max,
                    )
                    di += 1
                    if last and di == 4:
                        bits_sum(bits, 0, 2048, 0, b)
            if last:
                bits_sum(bits, 2048, 3584, 1, b)
            else:
                bits_sum(bits, 0, 3584, 0, b)

        nc.sync.dma_start(out_d.ap(), res[:])

    with tile.TileContext(nc) as tc, ExitStack() as ctx:
        emit(tc, ctx)

    nc.compile()
    return nc


def _get_nc(**kw):
    key = repr(sorted(kw.items()))
    if key not in _CACHE:
        _CACHE[key] = _build(**kw)
    return _CACHE[key]


def _make_in_maps(X, y):
    import ml_dtypes

    bf = ml_dtypes.bfloat16
    X = np.ascontiguousarray(np.asarray(X, dtype=np.float32))
    assert X.shape == (N, C)

    xt2 = np.ascontiguousarray((X.T / np.float32(TAU)).astype(bf))
    eyeA = np.concatenate([np.eye(C) * -1e4, np.zeros((C, C))], axis=1)
    eyeB = np.concatenate([np.zeros((C, C)), np.eye(C) * -1e4], axis=1)
    eyeneg = np.concatenate([eyeA, eyeB], axis=1).astype(bf)   # [64, 256]
    idn64d = np.eye(C).astype(bf)
    ose_pad = np.concatenate(
        [np.concatenate([eyeneg, idn64d], 1), np.zeros((C, 2 * P + C))],
        0).astype(bf)                               # [128, 320]

    in_maps = []
    for r in range(NCORES):
        rows = slice(r * RPC, (r + 1) * RPC)
        xbt = X[rows].T.astype(bf)
        xbtd = np.concatenate([xbt, xbt], 0)        # [128, 1024]
        xt2r = np.roll(xt2, -r * RPC, axis=1)
        xt2dd = np.concatenate([xt2r, xt2r], 0)     # [128, 8192]
        in_maps.append(
            {
                "headb": np.ascontiguousarray(np.concatenate(
                    [xbtd, ose_pad, xt2dd[:, :1536]], 1)),
                "xt2d": np.ascontiguousarray(xt2dd[:, 1536:]),
            }
        )
    return in_maps


def run(input, target, trace=False, **build_kw):
    """Run the device kernel; returns (loss_scalar, BassKernelResults)."""
    from concourse.bass_utils import run_bass_kernel_spmd

    X = np.ascontiguousarray(np.asarray(input, dtype=np.float32))
    y = np.asarray(target).astype(np.int64).ravel()

    nc = _get_nc(**build_kw)
    in_maps = _make_in_maps(X, y)
    res = run_bass_kernel_spmd(
        nc, in_maps, core_ids=list(range(NCORES)), trace=trace
    )

    # ---- host side: O(N*C) math in f64 ----
    X64 = X.astype(np.float64)
    m = X64.max(1)
    lseC = m + np.log(np.exp(X64 - m[:, None]).sum(1))
    ce = (lseC - X64[np.arange(N), y]).mean()

    cnt = np.bincount(y, minlength=C).astype(np.float64)
    S = (y[:, None] == np.arange(C)[None, :]).astype(np.float64).T @ X64
    npos = cnt[y] - 1.0
    possum = ((X64 * S[y]).sum(1) - (X64 * X64).sum(1)) / TAU
    pt = np.where(npos > 0, possum / np.maximum(npos, 1.0), 0.0)

    # per-row exp sums from the device partials
    se = np.empty(N, dtype=np.float64)
    for r, core_out in enumerate(res.results):
        o = core_out["out"].astype(np.float64)              # [128, 40]
        seaA = o[:, :O_ED].reshape(P, NBLK, N_ACT).sum(2)   # [128, 8]
        seaD = o[:, O_ED:].reshape(P, NPIECE, NBLK).sum(1)  # [128, 8]
        se_rows = seaA + SCALE48 * seaD                     # [p, b]
        # row (r*1024 + b*128 + p) <-> se_rows[p, b]
        se[r * RPC : (r + 1) * RPC] = se_rows.T.ravel()

    lse = np.log(se) + SH
    per = np.where(npos > 0, lse - pt, 0.0)
    sc = per.sum()
    loss = (1.0 - LMBD) * ce + LMBD * sc
    return np.array(loss, dtype=np.float32), res


def kernel(input, target):
    loss, _ = run(input, target, trace=False)
    return loss


# revision 37
# speedup vs baseline: 1.0032x; 1.0032x over previous
"""Fused CE + supervised-contrastive loss on 8 Trainium2 NeuronCores.

Math (reference semantics):
  ce   = -mean_i log_softmax(input)[i, y_i]
  sim  = (X @ X.T) / tau, diag excluded
  lse_i = logsumexp_{k!=i} sim[i,k]
  possum_i = (x_i . S_{y_i} - ||x_i||^2)/tau,  S_c = sum_{k: y_k=c} x_k
  per_i = lse_i - possum_i/n_pos_i  (0 if n_pos_i == 0)
  loss = (1-lmbd)*ce + lmbd * sum_i per_i

Only the O(N^2) term (the per-row exp-sums se_i = sum_k e^{sim-40})
runs on device; everything O(N*C) -- the CE term, class sums /
positive-pair dot products, and the final ln(se)/per_i combine -- is
cheap host numpy.  Each core owns 1024 rows (batch shard) and holds the
full X^T (replicated, bf16) so the sim block needs no communication
(the ncfw AllReduce measures 44us trigger-to-start latency on this
platform -- longer than the kernel -- so no collective is used).

The N^2 elements must leave PSUM through the only two engines with PSUM
read ports (ACT + DVE), so the kernel is drain-bound:
  - PE: sim matmuls emitted as 64x128 row-tile PAIRS, tile_position
    (0,0)/(64,0) alternating; disjoint row-groups execute concurrently.
  - ACT drains 3x[128,1536] chunks per block: exp(s-40) with fused
    accum_out row sums.
  - DVE drains 7x[128,512] chunks per block with a bit-hack exp:
    u16 = max(A*s, 0) gives the bf16 bits of ~e^(s - 127*ln2); the
    max-with-0 zeroes the -1e4 diag spikes and the s<0 tail (true
    contributions < e^-88: exactly the bf16 underflow).
  - The bits second pass: DVE pairwise bf16 folds (2x mode) down to
    448 cols, then one small Copy+accum on ACT.
Input DMAs: a single "head bundle" (xbtd | diag operands | xt2d cols
0:1536) lands first so matmuls start as early as possible.

Self-exclusion: X^T is rotated by -1024*core so row p of block b has
its self-column at local column b*128+p; two 64x128-tiled
accumulate-matmuls add -1e4 there before the drain (pure SPMD).

Outputs per core: [128, 40] f32 = per-(row-block, chunk) partial exp
sums (3 ACT slots + 2 bits slots per block).  Host combines in f64.
"""

import numpy as np

N, C = 8192, 64
NCORES = 8
RPC = N // NCORES          # rows per core (1024)
P = 128                    # partitions per row-block
NBLK = RPC // P            # 8 row blocks per core
TAU = 0.5
LMBD = 0.5

SH = 40.0                          # ACT-chunk logsumexp shift
A_EXP = 128.0 / float(np.log(2.0))         # 184.6627...
# DVE chunks compute max(A*s, 0) -> bf16 bits of e^(s - 127*ln2), i.e. an
# implied shift of 88.03; SCALE48 rescales their row sums to the ACT shift
# (the /1.042 centers the piecewise-linear 2^x hack's +0..8.6% bias).
SH_DVE = 127.0 * float(np.log(2.0))
SCALE48 = float(np.exp(SH_DVE - SH) / 1.042)

# per-block drain plan: (kind, start_col, width); A->ACT exp+accum,
# D->DVE bit-hack exp.  A: 3x1536, D: 7x512 -> 8192 total.
CHUNK_PLAN = [
    ("A", 0, 1536), ("D", 4608, 512), ("D", 5120, 512),
    ("A", 1536, 1536), ("D", 5632, 512), ("D", 6144, 512),
    ("A", 3072, 1536), ("D", 6656, 512), ("D", 7168, 512),
    ("D", 7680, 512),
]
# last block: A chunks first so ACT's final drain overlaps the D tail;
# bits folded in two pieces so the serial tail is small.
CHUNK_PLAN_LAST = [
    ("A", 0, 1536), ("A", 1536, 1536), ("A", 3072, 1536),
    ("D", 4608, 512), ("D", 5120, 512), ("D", 5632, 512),
    ("D", 6144, 512), ("D", 6656, 512), ("D", 7168, 512),
    ("D", 7680, 512),
]
N_ACT = sum(1 for k, _, _ in CHUNK_PLAN if k == "A")   # 3
N_DVE = sum(1 for k, _, _ in CHUNK_PLAN if k == "D")   # 7
NPIECE = 2
O_ED = NBLK * N_ACT        # esumD slot base inside the out tile (24)
OUT_W = O_ED + NPIECE * NBLK   # 40

_CACHE = {}


def _build():
    from contextlib import ExitStack

    import concourse.bass as bass
    import concourse.tile as tile
    from concourse import bacc, mybir

    f32 = mybir.dt.float32
    u16 = mybir.dt.uint16
    bf16 = mybir.dt.bfloat16
    AF = mybir.ActivationFunctionType
    ALU = mybir.AluOpType

    nc = bacc.Bacc(
        "TRN2",
        target_bir_lowering=False,
        debug=False,
        num_devices=NCORES,
    )

    # head bundle: xbtd | eyeneg+idn64 (padded to 128 rows) | xt2d[:, 0:1536]
    # -- everything block 0's first chunk needs, in ONE dma so the first
    # matmul isn't gated on multiple serialized transfers.
    HB_OSE = RPC                       # 1024
    HB_XT = RPC + 2 * P + C            # 1344
    HB_W = HB_XT + 1536                # 2880
    hd_d = nc.dram_tensor("headb", [P, HB_W], bf16, kind="ExternalInput")
    # xt2d columns 1536:8192
    xt2d_d = nc.dram_tensor("xt2d", [P, N - 1536], bf16, kind="ExternalInput")
    out_d = nc.dram_tensor("out", [P, OUT_W], f32, kind="ExternalOutput")

    def emit(tc, ctx):
        const = ctx.enter_context(tc.tile_pool(name="const", bufs=1))
        psA = ctx.enter_context(tc.tile_pool(name="psA", bufs=2, space="PSUM"))
        psD = ctx.enter_context(tc.tile_pool(name="psD", bufs=2, space="PSUM"))
        bitsp = ctx.enter_context(tc.tile_pool(name="bits", bufs=2))
        scr = ctx.enter_context(tc.tile_pool(name="scr", bufs=2))
        stats = ctx.enter_context(tc.tile_pool(name="stats", bufs=1))

        # ---- input DMAs: head bundle first, the rest behind it ----
        hd_sb = const.tile([P, HB_W], bf16)
        nc.sync.dma_start(hd_sb[:], hd_d.ap())
        O_EYE, O_IDN = HB_OSE, HB_OSE + 2 * P
        xt2d_sb = const.tile([P, N - 1536], bf16)
        nc.sync.dma_start(xt2d_sb[:, 0:3072], xt2d_d.ap()[:, 0:3072])
        nc.sync.dma_start(xt2d_sb[:, 3072:6656], xt2d_d.ap()[:, 3072:6656])

        def rhs_ap(h, c0, w):
            """xt2d columns c0:c0+w live in the head bundle (<1536) or the
            xt2d tile (>=1536); chunk pieces never straddle the boundary."""
            if c0 < 1536:
                assert c0 + w <= 1536
                return hd_sb[h : h + 64, HB_XT + c0 : HB_XT + c0 + w]
            return xt2d_sb[h : h + 64, c0 - 1536 : c0 - 1536 + w]

        # ---- persistent stat tiles ----
        nshift = stats.tile([P, 1], f32)
        nc.vector.memset(nshift[:], -SH)
        res = stats.tile([P, OUT_W], f32)
        nc.vector.memset(res[:, O_ED:OUT_W], 0.0)

        # ---- the O(N^2) drain ----
        toggle = [0]

        def sim_mms(ps, b, c0, width):
            """row-tiled sim matmuls: 512-col pieces, alternating PE halves."""
            lo = b * P
            for j in range(width // 512):
                h = 64 * toggle[0]
                toggle[0] ^= 1
                nc.tensor.matmul(
                    ps[:, j * 512 : (j + 1) * 512],
                    lhsT=hd_sb[h : h + 64, lo : lo + P],
                    rhs=rhs_ap(h, c0 + j * 512, 512),
                    start=True,
                    stop=True,
                )

        def diag_mms(ps, b):
            """kill self-similarity: -1e4 onto local cols b*128..+127.
            Both pieces on tile (0,0) so they serialize (concurrent row-tiles
            must not write the same PSUM bank)."""
            d0 = b * P
            nc.tensor.matmul(
                ps[:, d0 : d0 + 64],
                lhsT=hd_sb[0:C, O_EYE : O_EYE + P],
                rhs=hd_sb[0:C, O_IDN : O_IDN + C],
                start=False, stop=True, skip_group_check=True,
            )
            nc.tensor.matmul(
                ps[:, d0 + 64 : d0 + P],
                lhsT=hd_sb[0:C, O_EYE + P : O_EYE + 2 * P],
                rhs=hd_sb[0:C, O_IDN : O_IDN + C],
                start=False, stop=True, skip_group_check=True,
            )

        def bits_sum(bits, lo, hi, slot, b, stop_w=896):
            """fold bits[lo:hi] pairwise (bf16 TT, 2x mode) down to stop_w,
            then Copy+accum the remnant on ACT into the esumD slot.  The
            wider 896 remnant trades +373ns of ACT (the shorter queue) for
            -400ns of DVE (the critical queue) per block."""
            w = hi - lo
            cur, c0 = bits, lo
            while w > stop_w:
                nxt = scr.tile([P, w // 2], bf16, tag="fold")
                nc.vector.tensor_add(
                    nxt[:], cur[:, c0 : c0 + w // 2], cur[:, c0 + w // 2 : c0 + w]
                )
                cur, c0, w = nxt, 0, w // 2
            t = scr.tile([P, w], bf16, tag="crdump")
            col = O_ED + slot * NBLK + b
            nc.scalar.activation(
                t[:], cur[:, c0 : c0 + w], AF.Copy,
                accum_out=res[:, col : col + 1],
            )

        for b in range(NBLK):
            last = b == NBLK - 1
            di = 0
            bits = bitsp.tile([P, N_DVE * 512], bf16, tag="bits")
            plan = CHUNK_PLAN_LAST if last else CHUNK_PLAN
            for kind, c0, width in plan:
                if kind == "A":
                    ps = psA.tile([P, 1536], f32, tag="ps")
                    sim_mms(ps, b, c0, width)
                    if c0 == 0:
                        diag_mms(ps, b)
                    aidx = c0 // 1536
                    nc.scalar.activation(
                        ps[:], ps[:], AF.Exp, bias=nshift[:],
                        accum_out=res[:, b * N_ACT + aidx : b * N_ACT + aidx + 1],
                    )
                else:
                    ps = psD.tile([P, 512], f32, tag="ps")
                    sim_mms(ps, b, c0, 512)
                    nc.vector.tensor_scalar(
                        out=bits[:, di * 512 : (di + 1) * 512].bitcast(u16),
                        in0=ps[:],
                        scalar1=A_EXP, scalar2=0.0,
                        op0=ALU.mult, op1=ALU.max,
                    )
                    di += 1
                    if last and di == 4:
                        bits_sum(bits, 0, 2048, 0, b, stop_w=512)
            if last:
                bits_sum(bits, 2048, 3584, 1, b, stop_w=512)
            else:
                bits_sum(bits, 0, 3584, 0, b)

        nc.sync.dma_start(out_d.ap(), res[:])

    with tile.TileContext(nc) as tc, ExitStack() as ctx:
        emit(tc, ctx)

    nc.compile()
    return nc


def _get_nc(**kw):
    key = repr(sorted(kw.items()))
    if key not in _CACHE:
        _CACHE[key] = _build(**kw)
    return _CACHE[key]


def _make_in_maps(X, y):
    import ml_dtypes

    bf = ml_dtypes.bfloat16
    X = np.ascontiguousarray(np.asarray(X, dtype=np.float32))
    assert X.shape == (N, C)

    xt2 = np.ascontiguousarray((X.T / np.float32(TAU)).astype(bf))
    eyeA = np.concatenate([np.eye(C) * -1e4, np.zeros((C, C))], axis=1)
    eyeB = np.concatenate([np.zeros((C, C)), np.eye(C) * -1e4], axis=1)
    eyeneg = np.concatenate([eyeA, eyeB], axis=1).astype(bf)   # [64, 256]
    idn64d = np.eye(C).astype(bf)
    ose_pad = np.concatenate(
        [np.concatenate([eyeneg, idn64d], 1), np.zeros((C, 2 * P + C))],
        0).astype(bf)                               # [128, 320]

    in_maps = []
    for r in range(NCORES):
        rows = slice(r * RPC, (r + 1) * RPC)
        xbt = X[rows].T.astype(bf)
        xbtd = np.concatenate([xbt, xbt], 0)        # [128, 1024]
        xt2r = np.roll(xt2, -r * RPC, axis=1)
        xt2dd = np.concatenate([xt2r, xt2r], 0)     # [128, 8192]
        in_maps.append(
            {
                "headb": np.ascontiguousarray(np.concatenate(
                    [xbtd, ose_pad, xt2dd[:, :1536]], 1)),
                "xt2d": np.ascontiguousarray(xt2dd[:, 1536:]),
            }
        )
    return in_maps


def run(input, target, trace=False, **build_kw):
    """Run the device kernel; returns (loss_scalar, BassKernelResults)."""
    from concourse.bass_utils import run_bass_kernel_spmd

    X = np.ascontiguousarray(np.asarray(input, dtype=np.float32))
    y = np.asarray(target).astype(np.int64).ravel()

    nc = _get_nc(**build_kw)
    in_maps = _make_in_maps(X, y)
    res = run_bass_kernel_spmd(
        nc, in_maps, core_ids=list(range(NCORES)), trace=trace
    )

    # ---- host side: O(N*C) math in f64 ----
    X64 = X.astype(np.float64)
    m = X64.max(1)
    lseC = m + np.log(np.exp(X64 - m[:, None]).sum(1))
    ce = (lseC - X64[np.arange(N), y]).mean()

    cnt = np.bincount(y, minlength=C).astype(np.float64)
    S = (y[:, None] == np.arange(C)[None, :]).astype(np.float64).T @ X64
    npos = cnt[y] - 1.0
    possum = ((X64 * S[y]).sum(1) - (X64 * X64).sum(1)) / TAU
    pt = np.where(npos > 0, possum / np.maximum(npos, 1.0), 0.0)

    # per-row exp sums from the device partials
    se = np.empty(N, dtype=np.float64)
    for r, core_out in enumerate(res.results):
        o = core_out["out"].astype(np.float64)              # [128, 40]
        seaA = o[:, :O_ED].reshape(P, NBLK, N_ACT).sum(2)   # [128, 8]
        seaD = o[:, O_ED:].reshape(P, NPIECE, NBLK).sum(1)  # [128, 8]
        se_rows = seaA + SCALE48 * seaD                     # [p, b]
        # row (r*1024 + b*128 + p) <-> se_rows[p, b]
        se[r * RPC : (r + 1) * RPC] = se_rows.T.ravel()

    lse = np.log(se) + SH
    per = np.where(npos > 0, lse - pt, 0.0)
    sc = per.sum()
    loss = (1.0 - LMBD) * ce + LMBD * sc
    return np.array(loss, dtype=np.float32), res


def kernel(input, target):
    loss, _ = run(input, target, trace=False)
    return loss


# revision 39
# speedup vs baseline: 1.0071x; 1.0039x over previous
"""Fused CE + supervised-contrastive loss on 8 Trainium2 NeuronCores.

Math (reference semantics):
  ce   = -mean_i log_softmax(input)[i, y_i]
  sim  = (X @ X.T) / tau, diag excluded
  lse_i = logsumexp_{k!=i} sim[i,k]
  possum_i = (x_i . S_{y_i} - ||x_i||^2)/tau,  S_c = sum_{k: y_k=c} x_k
  per_i = lse_i - possum_i/n_pos_i  (0 if n_pos_i == 0)
  loss = (1-lmbd)*ce + lmbd * sum_i per_i

Only the O(N^2) term (the per-row exp-sums se_i = sum_k e^{sim-40})
runs on device; everything O(N*C) -- the CE term, class sums /
positive-pair dot products, and the final ln(se)/per_i combine -- is
cheap host numpy.  Each core owns 1024 rows (batch shard) and holds the
full X^T (replicated, bf16) so the sim block needs no communication
(the ncfw AllReduce measures 44us trigger-to-start latency on this
platform -- longer than the kernel -- so no collective is used).

The N^2 elements must leave PSUM through the only two engines with PSUM
read ports (ACT + DVE), so the kernel is drain-bound:
  - PE: sim matmuls emitted as 64x128 row-tile PAIRS, tile_position
    (0,0)/(64,0) alternating; disjoint row-groups execute concurrently.
  - ACT drains 3x[128,1536] chunks per block: exp(s-40) with fused
    accum_out row sums.
  - DVE drains 7x[128,512] chunks per block with a bit-hack exp:
    u16 = max(A*s, 0) gives the bf16 bits of ~e^(s - 127*ln2); the
    max-with-0 zeroes the -1e4 diag spikes and the s<0 tail (true
    contributions < e^-88: exactly the bf16 underflow).
  - The bits second pass: DVE pairwise bf16 folds (2x mode) down to
    448 cols, then one small Copy+accum on ACT.
Input DMAs: a single "head bundle" (xbtd | diag operands | xt2d cols
0:1536) lands first so matmuls start as early as possible.

Self-exclusion: X^T is rotated by -1024*core so row p of block b has
its self-column at local column b*128+p; two 64x128-tiled
accumulate-matmuls add -1e4 there before the drain (pure SPMD).

Outputs per core: [128, 40] f32 = per-(row-block, chunk) partial exp
sums (3 ACT slots + 2 bits slots per block).  Host combines in f64.
"""

import numpy as np

N, C = 8192, 64
NCORES = 8
RPC = N // NCORES          # rows per core (1024)
P = 128                    # partitions per row-block
NBLK = RPC // P            # 8 row blocks per core
TAU = 0.5
LMBD = 0.5

SH = 40.0                          # ACT-chunk logsumexp shift
A_EXP = 128.0 / float(np.log(2.0))         # 184.6627...
# DVE chunks compute max(A*s, 0) -> bf16 bits of e^(s - 127*ln2), i.e. an
# implied shift of 88.03; SCALE48 rescales their row sums to the ACT shift
# (the /1.042 centers the piecewise-linear 2^x hack's +0..8.6% bias).
SH_DVE = 127.0 * float(np.log(2.0))
SCALE48 = float(np.exp(SH_DVE - SH) / 1.042)

# per-block drain plan: (kind, start_col, width); A->ACT exp+accum,
# D->DVE bit-hack exp.  A: 3x1536, D: 7x512 -> 8192 total.
CHUNK_PLAN = [
    ("A", 0, 1536), ("D", 4608, 512), ("D", 5120, 512),
    ("A", 1536, 1536), ("D", 5632, 512), ("D", 6144, 512),
    ("A", 3072, 1536), ("D", 6656, 512), ("D", 7168, 512),
    ("D", 7680, 512),
]
# last block: A chunks first so ACT's final drain overlaps the D tail;
# bits folded in two pieces so the serial tail is small.
CHUNK_PLAN_LAST = [
    ("A", 0, 1536), ("A", 1536, 1536), ("A", 3072, 1536),
    ("D", 4608, 512), ("D", 5120, 512), ("D", 5632, 512),
    ("D", 6144, 512), ("D", 6656, 512), ("D", 7168, 512),
    ("D", 7680, 512),
]
N_ACT = sum(1 for k, _, _ in CHUNK_PLAN if k == "A")   # 3
N_DVE = sum(1 for k, _, _ in CHUNK_PLAN if k == "D")   # 7
NPIECE = 2
O_ED = NBLK * N_ACT        # esumD slot base inside the out tile (24)
OUT_W = O_ED + NPIECE * NBLK   # 40

_CACHE = {}


def _build():
    from contextlib import ExitStack

    import concourse.bass as bass
    import concourse.tile as tile
    from concourse import bacc, mybir

    f32 = mybir.dt.float32
    u16 = mybir.dt.uint16
    bf16 = mybir.dt.bfloat16
    AF = mybir.ActivationFunctionType
    ALU = mybir.AluOpType

    nc = bacc.Bacc(
        "TRN2",
        target_bir_lowering=False,
        debug=False,
        num_devices=NCORES,
    )

    # head bundle: xbtd | eyeneg+idn64 (padded to 128 rows) | xt2d[:, 0:1536]
    # -- everything block 0's first chunk needs, in ONE dma so the first
    # matmul isn't gated on multiple serialized transfers.
    HB_OSE = RPC                       # 1024
    HB_XT = RPC + 2 * P + C            # 1344
    HB_W = HB_XT + 1536                # 2880
    hd_d = nc.dram_tensor("headb", [P, HB_W], bf16, kind="ExternalInput")
    # xt2d columns 1536:8192
    xt2d_d = nc.dram_tensor("xt2d", [P, N - 1536], bf16, kind="ExternalInput")
    out_d = nc.dram_tensor("out", [P, OUT_W], f32, kind="ExternalOutput")

    def emit(tc, ctx):
        const = ctx.enter_context(tc.tile_pool(name="const", bufs=1))
        psA = ctx.enter_context(tc.tile_pool(name="psA", bufs=2, space="PSUM"))
        psD = ctx.enter_context(tc.tile_pool(name="psD", bufs=2, space="PSUM"))
        bitsp = ctx.enter_context(tc.tile_pool(name="bits", bufs=2))
        scr = ctx.enter_context(tc.tile_pool(name="scr", bufs=2))
        stats = ctx.enter_context(tc.tile_pool(name="stats", bufs=1))

        # ---- input DMAs: head bundle first, the rest behind it ----
        hd_sb = const.tile([P, HB_W], bf16)
        nc.sync.dma_start(hd_sb[:], hd_d.ap())
        O_EYE, O_IDN = HB_OSE, HB_OSE + 2 * P
        xt2d_sb = const.tile([P, N - 1536], bf16)
        nc.sync.dma_start(xt2d_sb[:, 0:3072], xt2d_d.ap()[:, 0:3072])
        nc.sync.dma_start(xt2d_sb[:, 3072:6656], xt2d_d.ap()[:, 3072:6656])

        def rhs_ap(h, c0, w):
            """xt2d columns c0:c0+w live in the head bundle (<1536) or the
            xt2d tile (>=1536); chunk pieces never straddle the boundary."""
            if c0 < 1536:
                assert c0 + w <= 1536
                return hd_sb[h : h + 64, HB_XT + c0 : HB_XT + c0 + w]
            return xt2d_sb[h : h + 64, c0 - 1536 : c0 - 1536 + w]

        # ---- persistent stat tiles ----
        nshift = stats.tile([P, 1], f32)
        nc.vector.memset(nshift[:], -SH)
        res = stats.tile([P, OUT_W], f32)
        nc.vector.memset(res[:, O_ED:OUT_W], 0.0)

        # ---- the O(N^2) drain ----
        toggle = [0]

        def sim_mms(ps, b, c0, width):
            """row-tiled sim matmuls: 512-col pieces, alternating PE halves."""
            lo = b * P
            for j in range(width // 512):
                h = 64 * toggle[0]
                toggle[0] ^= 1
                nc.tensor.matmul(
                    ps[:, j * 512 : (j + 1) * 512],
                    lhsT=hd_sb[h : h + 64, lo : lo + P],
                    rhs=rhs_ap(h, c0 + j * 512, 512),
                    start=True,
                    stop=True,
                )

        def diag_mms(ps, b):
            """kill self-similarity: -1e4 onto local cols b*128..+127.
            Both pieces on tile (0,0) so they serialize (concurrent row-tiles
            must not write the same PSUM bank)."""
            d0 = b * P
            nc.tensor.matmul(
                ps[:, d0 : d0 + 64],
                lhsT=hd_sb[0:C, O_EYE : O_EYE + P],
                rhs=hd_sb[0:C, O_IDN : O_IDN + C],
                start=False, stop=True, skip_group_check=True,
            )
            nc.tensor.matmul(
                ps[:, d0 + 64 : d0 + P],
                lhsT=hd_sb[0:C, O_EYE + P : O_EYE + 2 * P],
                rhs=hd_sb[0:C, O_IDN : O_IDN + C],
                start=False, stop=True, skip_group_check=True,
            )

        def bits_sum(bits, lo, hi, slot, b, stop_w=896):
            """fold bits[lo:hi] pairwise (bf16 TT, 2x mode) down to stop_w,
            then Copy+accum the remnant on ACT into the esumD slot.  The
            wider 896 remnant trades +373ns of ACT (the shorter queue) for
            -400ns of DVE (the critical queue) per block."""
            w = hi - lo
            cur, c0 = bits, lo
            while w > stop_w:
                nxt = scr.tile([P, w // 2], bf16, tag="fold")
                nc.vector.tensor_add(
                    nxt[:], cur[:, c0 : c0 + w // 2], cur[:, c0 + w // 2 : c0 + w]
                )
                cur, c0, w = nxt, 0, w // 2
            t = scr.tile([P, w], bf16, tag="crdump")
            col = O_ED + slot * NBLK + b
            nc.scalar.activation(
                t[:], cur[:, c0 : c0 + w], AF.Copy,
                accum_out=res[:, col : col + 1],
            )

        for b in range(NBLK):
            last = b == NBLK - 1
            di = 0
            bits = bitsp.tile([P, N_DVE * 512], bf16, tag="bits")
            plan = CHUNK_PLAN_LAST if last else CHUNK_PLAN
            for kind, c0, width in plan:
                if kind == "A":
                    ps = psA.tile([P, 1536], f32, tag="ps")
                    sim_mms(ps, b, c0, width)
                    if c0 == 0:
                        diag_mms(ps, b)
                    aidx = c0 // 1536
                    nc.scalar.activation(
                        ps[:], ps[:], AF.Exp, bias=nshift[:],
                        accum_out=res[:, b * N_ACT + aidx : b * N_ACT + aidx + 1],
                    )
                else:
                    ps = psD.tile([P, 512], f32, tag="ps")
                    sim_mms(ps, b, c0, 512)
                    nc.vector.tensor_scalar(
                        out=bits[:, di * 512 : (di + 1) * 512].bitcast(u16),
                        in0=ps[:],
                        scalar1=A_EXP, scalar2=0.0,
                        op0=ALU.mult, op1=ALU.max,
                    )
                    di += 1
                    if last and di == 4:
                        bits_sum(bits, 0, 2048, 0, b, stop_w=512)
            if last:
                bits_sum(bits, 2048, 3584, 1, b, stop_w=512)
            else:
                bits_sum(bits, 0, 3584, 0, b)

        nc.sync.dma_start(out_d.ap(), res[:])

    with tile.TileContext(nc) as tc, ExitStack() as ctx:
        emit(tc, ctx)

    nc.compile()
    return nc


def _get_nc(**kw):
    key = repr(sorted(kw.items()))
    if key not in _CACHE:
        _CACHE[key] = _build(**kw)
    return _CACHE[key]


def _make_in_maps(X, y):
    import ml_dtypes

    bf = ml_dtypes.bfloat16
    X = np.ascontiguousarray(np.asarray(X, dtype=np.float32))
    assert X.shape == (N, C)

    xt2 = np.ascontiguousarray((X.T / np.float32(TAU)).astype(bf))
    eyeA = np.concatenate([np.eye(C) * -1e4, np.zeros((C, C))], axis=1)
    eyeB = np.concatenate([np.zeros((C, C)), np.eye(C) * -1e4], axis=1)
    eyeneg = np.concatenate([eyeA, eyeB], axis=1).astype(bf)   # [64, 256]
    idn64d = np.eye(C).astype(bf)
    ose_pad = np.concatenate(
        [np.concatenate([eyeneg, idn64d], 1), np.zeros((C, 2 * P + C))],
        0).astype(bf)                               # [128, 320]

    in_maps = []
    for r in range(NCORES):
        rows = slice(r * RPC, (r + 1) * RPC)
        xbt = X[rows].T.astype(bf)
        xbtd = np.concatenate([xbt, xbt], 0)        # [128, 1024]
        xt2r = np.roll(xt2, -r * RPC, axis=1)
        xt2dd = np.concatenate([xt2r, xt2r], 0)     # [128, 8192]
        in_maps.append(
            {
                "headb": np.ascontiguousarray(np.concatenate(
                    [xbtd, ose_pad, xt2dd[:, :1536]], 1)),
                "xt2d": np.ascontiguousarray(xt2dd[:, 1536:]),
            }
        )
    return in_maps


def run(input, target, trace=False, **build_kw):
    """Run the device kernel; returns (loss_scalar, BassKernelResults)."""
    from concourse.bass_utils import run_bass_kernel_spmd

    X = np.ascontiguousarray(np.asarray(input, dtype=np.float32))
    y = np.asarray(target).astype(np.int64).ravel()

    nc = _get_nc(**build_kw)
    in_maps = _make_in_maps(X, y)
    res = run_bass_kernel_spmd(
        nc, in_maps, core_ids=list(range(NCORES)), trace=trace
    )

    # ---- host side: O(N*C) math in f64 ----
    X64 = X.astype(np.float64)
    m = X64.max(1)
    lseC = m + np.log(np.exp(X64 - m[:, None]).sum(1))
    ce = (lseC - X64[np.arange(N), y]).mean()

    cnt = np.bincount(y, minlength=C).astype(np.float64)
    S = (y[:, None] == np.arange(C)[None, :]).astype(np.float64).T @ X64
    npos = cnt[y] - 1.0
    possum = ((X64 * S[y]).sum(1) - (X64 * X64).sum(1)) / TAU
    pt = np.where(npos > 0, possum / np.maximum(npos, 1.0), 0.0)

    # per-row exp sums from the device partials
    se = np.empty(N, dtype=np.float64)
    for r, core_out in enumerate(res.results):
        o = core_out["out"].astype(np.float64)              # [128, 40]
        seaA = o[:, :O_ED].reshape(P, NBLK, N_ACT).sum(2)   # [128, 8]
        seaD = o[:, O_ED:].reshape(P, NPIECE, NBLK).sum(1)  # [128, 8]
        se_rows = seaA + SCALE48 * seaD                     # [p, b]
        # row (r*1024 + b*128 + p) <-> se_rows[p, b]
        se[r * RPC : (r + 1) * RPC] = se_rows.T.ravel()

    lse = np.log(se) + SH
    per = np.where(npos > 0, lse - pt, 0.0)
    sc = per.sum()
    loss = (1.0 - LMBD) * ce + LMBD * sc
    return np.array(loss, dtype=np.float32), res


def kernel(input, target):
    loss, _ = run(input, target, trace=False)
    return loss
